# revision 3
# baseline (speedup 1.0000x reference)
"""Trainium2 Bass kernel for BaseTensorMemory (delta-rule tensor memory).

Computes, for full inputs queries/keys/values [B,S,D], M [D,D], z [D]:
  sigma_k = elu(keys)+1 ; existing = (sigma_k@M)/(sigma_k@z+eps)
  delta_m = clip(einsum('bsd,bse->de', sigma_k, values-existing)/(B*S), +-1)
  delta_z = sigma_k.sum((0,1))/B
  M' = clip(M+delta_m, +-100); z' = clip(z+delta_z, eps, 1e6)
  out = (sigma_q@M')/(sigma_q@z'+eps)

Strategy (v2): data-parallel over 8 NeuronCores, fp16 I/O (inputs cast on
host, output cast back; rel-err budget 2e-2 >> fp16 noise ~1e-3).

Gram-matrix restructure: the per-token "existing" tensor is never
materialized.  sigma_k^T @ existing == G @ M with G = sigma_k^T diag(rn)
sigma_k (rn = 1/(norm+eps)), a 64x64 Gram matrix accumulated on PSUM by
matmuls.  G@(-M) is folded into the delta accumulator with one tiny matmul
before the AllReduce, so the AR payload is [64,65] f32.  This kills the
baseline's values-relayout copy, the big "ex" multiply and the 129-wide
delta matmuls.

Block-diagonal rhs trick: transposed sigma tiles stack two token-groups on
the 128 partitions (d of group A on partitions 0:64, d of group B on
64:128).  Retrieve/norm matmuls use rhs = [[W;0],[0;W]] (W replicated on
both partition halves' diagonal blocks) so one k=128 matmul serves both
groups and no operand ever sits at base partition 64 alone (avoids the
PSUM accumulation-group base-alternation crash the baseline worked around
with parity banking).

elu(x)+1 == exp(min(x,0)) + relu(x) exactly: min/relu run as DVE
tensor_scalar ops (4x fp16 mode), exp on ACT, and the two legs are summed
by one fp16 2x tensor_tensor add (keys; feeds delta/G matmuls directly)
or on GPSIMD (queries) to keep DVE free for the output normalize.

Engine split per macro-tile: PE does transposes + all matmuls, ACT does
exp + PSUM->SBUF sigT copies, DVE does elu legs + reciprocal + normalize
multiplies, GPSIMD does the sigma*rn scaling (keys) / elu combine
(queries).  The AllReduce is overlapped with the first SKEW queries
fronts, as in v1.
"""

import numpy as np

B, S, D = 16, 16384, 64
N_CORES = 8
EPS = 1e-6
MAX_DELTA = 1.0
MAX_MEMORY = 100.0
MAX_NORM = 1e6

TILE_TOKENS = 2048  # macro-tile: [128, 1024] f16, two 1024-token halves
QPM = TILE_TOKENS // 128  # 16 token-groups per macro-tile


def _build(n_cores, tokens_per_core):
    import concourse.bacc as bacc
    import concourse.mybir as mybir
    import concourse.tile as tile
    from concourse import masks

    dt = mybir.dt
    f32, f16 = dt.float32, dt.float16
    A = mybir.AluOpType
    F = mybir.ActivationFunctionType

    T = tokens_per_core
    NT = T // TILE_TOKENS
    assert NT * TILE_TOKENS == T
    NH = 2 * NT
    MFD = QPM * D  # 1024

    nc = bacc.Bacc(
        "TRN2", target_bir_lowering=False, debug=False, num_devices=n_cores
    )
    k_d = nc.dram_tensor("keys", [T, D], f16, kind="ExternalInput").ap()
    v_d = nc.dram_tensor("values", [T, D], f16, kind="ExternalInput").ap()
    q_d = nc.dram_tensor("queries", [T, D], f16, kind="ExternalInput").ap()
    m_d = nc.dram_tensor("m", [D, D], f32, kind="ExternalInput").ap()
    z_d = nc.dram_tensor("z", [D, 1], f32, kind="ExternalInput").ap()
    o_d = nc.dram_tensor("out", [T, D], f16, kind="ExternalOutput").ap()

    kr = k_d.rearrange("(n p q) d -> n p (q d)", p=128, q=QPM)
    vr = v_d.rearrange("(n p q) d -> n p (q d)", p=128, q=QPM)
    qr = q_d.rearrange("(n p q) d -> n p (q d)", p=128, q=QPM)
    orr = o_d.rearrange("(n p q) d -> n p (q d)", p=128, q=QPM)

    with tile.TileContext(nc) as tc:
        with (
            tc.tile_pool(name="const", bufs=1) as cpool,
            tc.tile_pool(name="io", bufs=3) as io,
            tc.tile_pool(name="work", bufs=3) as work,
            tc.tile_pool(name="szt", bufs=3) as szt,
            tc.tile_pool(name="srn", bufs=3) as srnp,
            tc.tile_pool(name="small", bufs=4) as small,
            tc.tile_pool(name="sigq", bufs=20) as sigq,
            tc.tile_pool(name="psT", bufs=2, space="PSUM") as psTp,
            tc.tile_pool(name="psN", bufs=2, space="PSUM") as psNp,
            tc.tile_pool(name="psR", bufs=2, space="PSUM") as psRp,
            tc.tile_pool(name="psA", bufs=1, space="PSUM") as psAp,
            tc.tile_pool(name="psG", bufs=1, space="PSUM") as psGp,
            tc.tile_pool(name="dram", bufs=1, space="DRAM") as dram,
        ):
            ident = cpool.tile([128, 128], f16)
            masks.make_identity(nc, ident[:])

            # M, z in f32 (partitions 0:64) for the update math.
            mz = cpool.tile([64, 65], f32)
            nc.sync.dma_start(mz[:, 0:64], m_d[:])
            nc.sync.dma_start(mz[:, 64:65], z_d[:])

            # Block-diagonal z for the keys norm matmuls: [[z,0],[0,z]].
            zstage = cpool.tile([128, 2], f32)
            nc.gpsimd.memset(zstage[:], 0.0)
            nc.sync.dma_start(zstage[0:64, 0:1], z_d[:])
            nc.sync.dma_start(zstage[64:128, 1:2], z_d[:])
            z2x16 = cpool.tile([128, 2], f16)
            nc.scalar.copy(z2x16[:], zstage[:])

            negM16 = cpool.tile([64, 64], f16)
            nc.scalar.mul(negM16[:], mz[:, 0:64], -1.0)

            ones16 = cpool.tile([128, 1], f16)
            nc.gpsimd.memset(ones16[:], 1.0)

            # Updated-state tiles (filled post-AllReduce); zero the
            # off-diagonal blocks once.
            Mn2x16 = cpool.tile([128, 128], f16)
            nc.gpsimd.memset(Mn2x16[:], 0.0)
            zn2x16 = cpool.tile([128, 2], f16)
            nc.gpsimd.memset(zn2x16[:], 0.0)
            mzn = cpool.tile([64, 65], f32)

            psA = psAp.tile([64, 65], f32)
            psG = psGp.tile([64, 64], f32)

            first_mm = [True]  # psA bank: start=True only on very first
            first_g = [True]
            ncopy = [0]

            def copy_sigT(psT, pool, tag, eng=None):
                sigT = pool.tile([128, 512], f16, tag=tag)
                if eng is None:
                    eng = "act" if ncopy[0] % 2 else "dve"
                    ncopy[0] += 1
                if eng == "act":
                    nc.scalar.copy(sigT[:], psT[:])
                else:
                    nc.vector.tensor_copy(sigT[:], psT[:])
                return sigT

            def transpose_half(sig, a, pool, tag, eng=None):
                psT = psTp.tile([128, 512], f16, tag="psT")
                for c in range(4):
                    nc.tensor.matmul(
                        psT[:, c * 128 : (c + 1) * 128],
                        sig[:, a * 512 + c * 128 : a * 512 + (c + 1) * 128],
                        ident[:],
                        is_transpose=True,
                        start=(c == 0),
                        stop=(c == 3),
                    )
                return copy_sigT(psT, pool, tag, eng)

            # ---------------- keys phase ----------------
            kst = {}  # macro i -> dict(sig, vt, psN, rn, sigT{h}, srn{h})

            def front_k(i):
                kt = io.tile([128, MFD], f16, tag="kt")
                nc.sync.dma_start(kt[:], kr[i])
                vt = io.tile([128, MFD], f16, tag="vt")
                nc.sync.dma_start(vt[:], vr[i])
                m16 = work.tile([128, MFD], f16, tag="m")
                nc.vector.tensor_scalar_min(m16[:], kt[:], 0.0)
                e16 = work.tile([128, MFD], f16, tag="e")
                nc.scalar.activation(e16[:], m16[:], F.Exp)
                r16 = work.tile([128, MFD], f16, tag="r")
                nc.vector.tensor_scalar_max(r16[:], kt[:], 0.0)
                sig = work.tile([128, MFD], f16, tag="sig")
                nc.vector.tensor_add(sig[:], e16[:], r16[:])
                kst[i] = {"sig": sig, "vt": vt}

            def trans_k(h):
                i, a = h // 2, h % 2
                st = kst[i]
                sigT = transpose_half(st["sig"], a, szt, "sigT", eng="act")
                st["sigT%d" % a] = sigT
                if a == 0:
                    st["psN"] = psNp.tile([128, 16], f32, tag="psN", name="psN")
                psN = st["psN"]
                for c in range(4):
                    nc.tensor.matmul(
                        psN[:, a * 8 + c * 2 : a * 8 + c * 2 + 2],
                        sigT[:, c * 128 : (c + 1) * 128],
                        z2x16[:],
                        start=(a == 0 and c == 0),
                        stop=(a == 1 and c == 3),
                    )

            def recip_k(i):
                st = kst[i]
                t0 = small.tile([128, 16], f32, tag="t0")
                nc.vector.tensor_scalar_add(t0[:], st["psN"][:], EPS)
                rn = small.tile([128, 16], f32, tag="rn")
                nc.vector.reciprocal(rn[:], t0[:])
                st["rn"] = rn

            def srn_k(h):
                i, a = h // 2, h % 2
                st = kst[i]
                srn = srnp.tile([128, 512], f16, tag="srn")
                sigv = (
                    st["sig"][:, a * 512 : (a + 1) * 512]
                    .rearrange("p (g c) -> p g c", g=8)
                )
                rnv = (
                    st["rn"][:, a * 8 : (a + 1) * 8]
                    .rearrange("p g -> p g", g=8)
                    .unsqueeze(2)
                    .broadcast_to((128, 8, 64))
                )
                srnv = srn[:].rearrange("p (g c) -> p g c", g=8)
                nc.gpsimd.tensor_tensor(srnv, sigv, rnv, op=A.mult)
                st["srn%d" % a] = srn

            def back_k(h, last):
                i, a = h // 2, h % 2
                st = kst[i]
                sig, vt, srn = st["sig"], st["vt"], st["srn%d" % a]
                for g in range(8):
                    q = a * 8 + g
                    sblk = sig[:, q * 64 : (q + 1) * 64]
                    nc.tensor.matmul(
                        psG[:],
                        srn[:, g * 64 : (g + 1) * 64],
                        sblk,
                        start=first_g[0],
                        stop=(last and g == 7),
                    )
                    first_g[0] = False
                    nc.tensor.matmul(
                        psA[:, 0:64],
                        sblk,
                        vt[:, q * 64 : (q + 1) * 64],
                        start=first_mm[0],
                        stop=False,
                    )
                    first_mm[0] = False
                    nc.tensor.matmul(
                        psA[:, 64:65], sblk, ones16[:], start=False, stop=False
                    )

            for h in range(NH):
                i, a = h // 2, h % 2
                if a == 0:
                    front_k(i)
                trans_k(h)
                if a == 1:
                    recip_k(i)
                if h >= 2:
                    srn_k(h - 2)
                    back_k(h - 2, last=False)
            srn_k(NH - 2)
            back_k(NH - 2, last=False)
            srn_k(NH - 1)
            back_k(NH - 1, last=True)

            # ---- fold G@(-M) into psA, then AllReduce [64,65] ----
            G16 = cpool.tile([64, 64], f16)
            nc.vector.tensor_copy(G16[:], psG[:])
            nc.tensor.matmul(
                psA[:, 0:64], G16[:], negM16[:], start=False, stop=True
            )
            accsb = cpool.tile([64, 65], f32)
            nc.vector.tensor_copy(accsb[:], psA[:])
            if n_cores > 1:
                arin = dram.tile([64, 65], f32)
                arout = dram.tile([64, 65], f32)
                nc.gpsimd.dma_start(arin[:], accsb[:])
                nc.gpsimd.collective_compute(
                    "AllReduce",
                    mybir.AluOpType.add,
                    replica_groups=[list(range(n_cores))],
                    ins=[arin.opt()],
                    outs=[arout.opt()],
                )
                arsb = cpool.tile([64, 65], f32)
            else:
                arsb = accsb

            def update_math():
                # delta_m = clip((sv - G@M)/(B*S), +-1); M' = clip(M+dm,+-100)
                nc.vector.tensor_scalar(
                    mzn[:, 0:64], arsb[:, 0:64], 1.0 / (B * S), MAX_DELTA,
                    op0=A.mult, op1=A.min,
                )
                nc.vector.scalar_tensor_tensor(
                    mzn[:, 0:64], mzn[:, 0:64], -MAX_DELTA, mz[:, 0:64],
                    op0=A.max, op1=A.add,
                )
                nc.vector.tensor_scalar(
                    mzn[:, 0:64], mzn[:, 0:64], MAX_MEMORY, -MAX_MEMORY,
                    op0=A.min, op1=A.max,
                )
                # delta_z = acc_z/B; z' = clip(z+dz, eps, 1e6)
                nc.vector.scalar_tensor_tensor(
                    mzn[:, 64:65], arsb[:, 64:65], 1.0 / B, mz[:, 64:65],
                    op0=A.mult, op1=A.add,
                )
                nc.vector.tensor_scalar(
                    mzn[:, 64:65], mzn[:, 64:65], EPS, MAX_NORM,
                    op0=A.max, op1=A.min,
                )

            def update_cast():
                nc.gpsimd.tensor_copy(Mn2x16[0:64, 0:64], mzn[:, 0:64])
                nc.gpsimd.tensor_copy(Mn2x16[64:128, 64:128], mzn[:, 0:64])
                nc.gpsimd.tensor_copy(zn2x16[0:64, 0:1], mzn[:, 64:65])
                nc.gpsimd.tensor_copy(zn2x16[64:128, 1:2], mzn[:, 64:65])

            # ---------------- queries phase ----------------
            qst = {}  # macro i -> dict(psN, rn, ot); sigTq in sigq pool
            sigTq = {}

            def front_q(h):
                i, a = h // 2, h % 2
                if a == 0:
                    qt = io.tile([128, MFD], f16, tag="kt")
                    nc.sync.dma_start(qt[:], qr[i])
                    m16 = work.tile([128, MFD], f16, tag="m")
                    nc.vector.tensor_scalar_min(m16[:], qt[:], 0.0)
                    e16 = work.tile([128, MFD], f16, tag="e")
                    nc.scalar.activation(e16[:], m16[:], F.Exp)
                    r16 = work.tile([128, MFD], f16, tag="r")
                    nc.vector.tensor_scalar_max(r16[:], qt[:], 0.0)
                    sig = work.tile([128, MFD], f16, tag="sig")
                    nc.gpsimd.tensor_tensor(sig[:], e16[:], r16[:], op=A.add)
                    qst[i] = {"sig": sig}
                sigTq[h] = transpose_half(qst[i]["sig"], a, sigq, "sigTq")

            def mid_q(h):
                i, a = h // 2, h % 2
                st = qst[i]
                sigT = sigTq[h]
                psR = psRp.tile([128, 512], f32, tag="psR", name="psR")
                st["psR%d" % a] = psR
                for c in range(4):
                    blk = sigT[:, c * 128 : (c + 1) * 128]
                    nc.tensor.matmul(
                        psR[:, c * 128 : (c + 1) * 128],
                        blk,
                        Mn2x16[:],
                        start=(c == 0),
                        stop=(c == 3),
                    )
                if a == 0:
                    st["psN"] = psNp.tile([128, 16], f32, tag="psN", name="psN")
                psN = st["psN"]
                for c in range(4):
                    nc.tensor.matmul(
                        psN[:, a * 8 + c * 2 : a * 8 + c * 2 + 2],
                        sigT[:, c * 128 : (c + 1) * 128],
                        zn2x16[:],
                        start=(a == 0 and c == 0),
                        stop=(a == 1 and c == 3),
                    )

            def om_q(h):
                i, a = h // 2, h % 2
                st = qst[i]
                if a == 0:
                    t0 = small.tile([128, 16], f32, tag="t0")
                    nc.vector.tensor_scalar_add(t0[:], st["psN"][:], EPS)
                    rn = small.tile([128, 16], f32, tag="rn")
                    nc.vector.reciprocal(rn[:], t0[:])
                    st["rn"] = rn
                    st["ot"] = io.tile([128, MFD], f16, tag="ot", name="ot")
                rn, ot = st["rn"], st["ot"]
                rnv = (
                    rn[:, a * 8 : (a + 1) * 8]
                    .rearrange("p g -> p g", g=8)
                    .unsqueeze(2)
                    .broadcast_to((128, 8, 64))
                )
                psRv = st["psR%d" % a][:].rearrange("p (g c) -> p g c", g=8)
                otv = ot[:, a * 512 : (a + 1) * 512].rearrange(
                    "p (g c) -> p g c", g=8
                )
                nc.vector.tensor_mul(otv, psRv, rnv)
                if a == 1:
                    nc.sync.dma_start(orr[i], ot[:])

            def macro_mid(j):
                mid_q(2 * j)
                mid_q(2 * j + 1)
                om_q(2 * j)
                om_q(2 * j + 1)

            SKEW = min(14, NH)
            for h in range(SKEW):
                front_q(h)
            if n_cores > 1:
                nc.gpsimd.dma_start(arsb[:], arout[:])
            update_math()
            update_cast()
            mids = 0
            for h in range(SKEW, NH):
                front_q(h)
                hh = h - SKEW
                if hh % 2 == 1:
                    macro_mid(hh // 2)
                    mids += 1
            for j in range(mids, NT):
                macro_mid(j)

    nc.compile()
    return nc


_CACHE = {}


def _get_kernel(n_cores, tokens_per_core):
    key = (n_cores, tokens_per_core)
    if key not in _CACHE:
        _CACHE[key] = _build(n_cores, tokens_per_core)
    return _CACHE[key]


def make_in_maps(queries, keys, values, M, z, n_cores=N_CORES):
    b, s, d = keys.shape
    tot = b * s
    tpc = tot // n_cores
    kf = np.ascontiguousarray(keys.reshape(tot, d), dtype=np.float16)
    vf = np.ascontiguousarray(values.reshape(tot, d), dtype=np.float16)
    qf = np.ascontiguousarray(queries.reshape(tot, d), dtype=np.float16)
    m32 = np.ascontiguousarray(M, dtype=np.float32)
    z32 = np.ascontiguousarray(z, dtype=np.float32).reshape(d, 1)
    in_maps = []
    for c in range(n_cores):
        sl = slice(c * tpc, (c + 1) * tpc)
        in_maps.append(
            {
                "keys": np.ascontiguousarray(kf[sl]),
                "values": np.ascontiguousarray(vf[sl]),
                "queries": np.ascontiguousarray(qf[sl]),
                "m": m32,
                "z": z32,
            }
        )
    return in_maps, tpc


def _np_reference(queries, keys, values, M, z):
    """Fallback (is_empty edge case) — straight numpy port of the reference."""

    def elu1(x):
        return np.where(x > 0, x + 1.0, np.exp(np.minimum(x, 0.0)))

    def retrieve(sig, M, z):
        return (sig @ M) / ((sig @ z)[..., None] + EPS)

    sk = elu1(keys)
    existing = retrieve(sk, M, z)
    uv = values if z.sum() == 0 else values - existing
    dm = np.clip(
        np.einsum("bsd,bse->de", sk, uv) / (B * S), -MAX_DELTA, MAX_DELTA
    )
    dz = sk.sum(axis=(0, 1)) / B
    Mn = np.clip(M + dm, -MAX_MEMORY, MAX_MEMORY)
    zn = np.clip(z + dz, EPS, MAX_NORM)
    return retrieve(elu1(queries), Mn, zn).astype(np.float32)


def kernel(queries, keys, values, M, z, _want_results_obj=False, **_ignored):
    from concourse import bass_utils

    queries = np.asarray(queries)
    keys = np.asarray(keys)
    values = np.asarray(values)
    M = np.ascontiguousarray(M, dtype=np.float32)
    z = np.ascontiguousarray(z, dtype=np.float32)

    if float(z.sum()) == 0.0:
        # is_empty branch of the reference: update_values = values. Rare
        # (z all-zero); handled on host rather than in the kernel.
        return _np_reference(
            np.asarray(queries, dtype=np.float32),
            np.asarray(keys, dtype=np.float32),
            np.asarray(values, dtype=np.float32),
            M,
            z,
        )

    b, s, d = keys.shape
    in_maps, tpc = make_in_maps(queries, keys, values, M, z)
    nc = _get_kernel(N_CORES, tpc)
    res = bass_utils.run_bass_kernel_spmd(
        nc, in_maps, core_ids=list(range(N_CORES))
    )
    out = np.concatenate(
        [res.results[c]["out"] for c in range(N_CORES)], axis=0
    ).astype(np.float32).reshape(b, s, d)
    if _want_results_obj:
        return out, res
    return out


# revision 8
# speedup vs baseline: 1.1420x; 1.1420x over previous
"""Trainium2 Bass kernel for BaseTensorMemory (delta-rule tensor memory).

Computes, for full inputs queries/keys/values [B,S,D], M [D,D], z [D]:
  sigma_k = elu(keys)+1 ; existing = (sigma_k@M)/(sigma_k@z+eps)
  delta_m = clip(einsum('bsd,bse->de', sigma_k, values-existing)/(B*S), +-1)
  delta_z = sigma_k.sum((0,1))/B
  M' = clip(M+delta_m, +-100); z' = clip(z+delta_z, eps, 1e6)
  out = (sigma_q@M')/(sigma_q@z'+eps)

Strategy (v3): data-parallel over 8 NeuronCores, fp16 I/O (inputs cast on
host, output cast back; rel-err budget 2e-2 >> fp16 noise ~1e-3).

Gram-matrix restructure: the per-token "existing" tensor is never
materialized.  sigma_k^T @ existing == G @ M with G = sigma_k^T diag(rn)
sigma_k (rn = 1/norm), a 64x64 Gram matrix accumulated on PSUM.  G@(-M) is
folded into the delta accumulator with one tiny matmul before the
AllReduce, so the AR payload is [64,65] f32.

Keys phase has NO transposes: the per-token norm sigma@z is computed on
DVE as one broadcast multiply against a partition-replicated z row plus a
per-group tensor_reduce, so keys PE work is only the delta/ones/G matmuls
(which share the sig-block stationary weights per token-group to avoid
weight-reload stalls).

Queries phase transposes sigma_q and retrieves with a block-diagonal rhs
[[M';0],[0;M']] so one k=128 matmul serves the two token-groups stacked on
the transposed tile's partitions (no PSUM base-alternation, no parity
banking).  The norm matmul (rhs=[[z';0],[0;z']], n=2) reuses the
just-loaded stationary weights of the retrieve matmul, making it ~free.

elu(x)+1 == min(exp(x),1) + relu(x): exp on ACT straight from the f16
input (safe: exp overflows f16 only for x>11.09, far outside this data's
range), relu on ACT, one DVE/GPSIMD scalar_tensor_tensor combines.  The
eps in the normalizer is dropped (norms are O(20)..O(1e5); eps=1e-6 is
~1e-8 relative).

The AllReduce is overlapped with the first SKEW queries fronts; those
fronts avoid GPSIMD entirely because the gpsimd queue blocks on the
collective until it completes (post-AR fronts alternate their elu combine
between DVE and GPSIMD to offload the vector engine).
"""

import numpy as np

B, S, D = 16, 16384, 64
N_CORES = 8
EPS = 1e-6
MAX_DELTA = 1.0
MAX_MEMORY = 100.0
MAX_NORM = 1e6

TILE_TOKENS = 2048  # macro-tile: [128, 1024] f16, two 1024-token halves
QPM = TILE_TOKENS // 128  # 16 token-groups per macro-tile


def _build(n_cores, tokens_per_core):
    import concourse.bacc as bacc
    import concourse.mybir as mybir
    import concourse.tile as tile
    from concourse import masks

    dt = mybir.dt
    f32, f16 = dt.float32, dt.float16
    A = mybir.AluOpType
    F = mybir.ActivationFunctionType
    X = mybir.AxisListType.X

    T = tokens_per_core
    NT = T // TILE_TOKENS
    assert NT * TILE_TOKENS == T
    NH = 2 * NT
    MFD = QPM * D  # 1024

    nc = bacc.Bacc(
        "TRN2", target_bir_lowering=False, debug=False, num_devices=n_cores
    )
    k_d = nc.dram_tensor("keys", [T, D], f16, kind="ExternalInput").ap()
    v_d = nc.dram_tensor("values", [T, D], f16, kind="ExternalInput").ap()
    q_d = nc.dram_tensor("queries", [T, D], f16, kind="ExternalInput").ap()
    m_d = nc.dram_tensor("m", [D, D], f32, kind="ExternalInput").ap()
    z_d = nc.dram_tensor("z", [D, 1], f32, kind="ExternalInput").ap()
    o_d = nc.dram_tensor("out", [T, D], f16, kind="ExternalOutput").ap()

    kr = k_d.rearrange("(n p q) d -> n p (q d)", p=128, q=QPM)
    vr = v_d.rearrange("(n p q) d -> n p (q d)", p=128, q=QPM)
    qr = q_d.rearrange("(n p q) d -> n p (q d)", p=128, q=QPM)
    orr = o_d.rearrange("(n p q) d -> n p (q d)", p=128, q=QPM)

    with tile.TileContext(nc) as tc:
        with (
            tc.tile_pool(name="const", bufs=1) as cpool,
            tc.tile_pool(name="io", bufs=3) as io,
            tc.tile_pool(name="work", bufs=3) as work,
            tc.tile_pool(name="srn", bufs=3) as srnp,
            tc.tile_pool(name="small", bufs=4) as small,
            tc.tile_pool(name="sigq", bufs=20) as sigq,
            tc.tile_pool(name="psT", bufs=2, space="PSUM") as psTp,
            tc.tile_pool(name="psN", bufs=2, space="PSUM") as psNp,
            tc.tile_pool(name="psR", bufs=2, space="PSUM") as psRp,
            tc.tile_pool(name="psA", bufs=1, space="PSUM") as psAp,
            tc.tile_pool(name="psG", bufs=1, space="PSUM") as psGp,
            tc.tile_pool(name="dram", bufs=1, space="DRAM") as dram,
        ):
            ident = cpool.tile([128, 128], f16)
            masks.make_identity(nc, ident[:])

            # M, z in f32 (partitions 0:64) for the update math.
            mz = cpool.tile([64, 65], f32)
            nc.sync.dma_start(mz[:, 0:64], m_d[:])
            nc.sync.dma_start(mz[:, 64:65], z_d[:])

            negM16 = cpool.tile([64, 64], f16)
            nc.scalar.mul(negM16[:], mz[:, 0:64], -1.0)

            ones16 = cpool.tile([128, 1], f16)
            nc.gpsimd.memset(ones16[:], 1.0)


            # z replicated along the free dim on all partitions (keys norm).
            zrow = cpool.tile([1, 64], f32)
            nc.sync.dma_start(zrow[:], z_d.rearrange("d c -> c d"))
            zrep32 = cpool.tile([128, 64], f32)
            nc.gpsimd.partition_broadcast(zrep32[:], zrow[:])
            zrep16 = cpool.tile([128, 64], f16)
            nc.scalar.copy(zrep16[:], zrep32[:])

            # Updated-state tiles (filled post-AllReduce); zero the
            # off-diagonal blocks once.
            Mn2x16 = cpool.tile([128, 128], f16)
            nc.gpsimd.memset(Mn2x16[:], 0.0)
            zn2x16 = cpool.tile([128, 2], f16)
            nc.gpsimd.memset(zn2x16[:], 0.0)
            mzn = cpool.tile([64, 65], f32)

            psA = psAp.tile([64, 65], f32)
            psG = psGp.tile([64, 64], f32)

            first_mm = [True]  # psA bank: start=True only on very first
            first_g = [True]
            ncopy = [0]

            # ---------------- keys phase ----------------
            kst = {}  # macro i -> dict(sig, vt, rn, srn{a})

            def front_k(i):
                kt = io.tile([128, MFD], f16, tag="kt")
                nc.sync.dma_start(kt[:], kr[i])
                vt = io.tile([128, MFD], f16, tag="vt")
                nc.sync.dma_start(vt[:], vr[i])
                e16 = work.tile([128, MFD], f16, tag="e")
                nc.scalar.activation(e16[:], kt[:], F.Exp)
                r16 = work.tile([128, MFD], f16, tag="r")
                nc.scalar.activation(r16[:], kt[:], F.Relu)
                sig = work.tile([128, MFD], f16, tag="sig")
                nc.vector.scalar_tensor_tensor(
                    sig[:], e16[:], 1.0, r16[:], op0=A.min, op1=A.add
                )
                # norm[t,g] = sum_d sig[t,g,d] * z[d]  (no transpose)
                nmt = work.tile([128, MFD], f16, tag="nmt")
                sigv = sig[:].rearrange("p (g c) -> p g c", g=QPM)
                zbc = zrep16[:].unsqueeze(1).broadcast_to((128, QPM, 64))
                nmtv = nmt[:].rearrange("p (g c) -> p g c", g=QPM)
                nc.vector.tensor_tensor(nmtv, sigv, zbc, op=A.mult)
                nrm = small.tile([128, QPM], f32, tag="nrm")
                nc.vector.tensor_reduce(nrm[:], nmtv, X, A.add)
                rn = small.tile([128, QPM], f32, tag="rn")
                nc.vector.reciprocal(rn[:], nrm[:])
                kst[i] = {"sig": sig, "vt": vt, "rn": rn}

            def bd_k(h):
                # delta_sv and delta_z matmuls; sig-block stationary reused.
                i, a = h // 2, h % 2
                st = kst[i]
                sig, vt = st["sig"], st["vt"]
                for g in range(8):
                    q = a * 8 + g
                    sblk = sig[:, q * 64 : (q + 1) * 64]
                    nc.tensor.matmul(
                        psA[:, 0:64],
                        sblk,
                        vt[:, q * 64 : (q + 1) * 64],
                        start=first_mm[0],
                        stop=False,
                    )
                    first_mm[0] = False
                    nc.tensor.matmul(
                        psA[:, 64:65], sblk, ones16[:], start=False, stop=False
                    )

            def srn_k(h):
                i, a = h // 2, h % 2
                st = kst[i]
                srn = srnp.tile([128, 512], f16, tag="srn")
                sigv = (
                    st["sig"][:, a * 512 : (a + 1) * 512]
                    .rearrange("p (g c) -> p g c", g=8)
                )
                rnv = (
                    st["rn"][:, a * 8 : (a + 1) * 8]
                    .unsqueeze(2)
                    .broadcast_to((128, 8, 64))
                )
                srnv = srn[:].rearrange("p (g c) -> p g c", g=8)
                nc.gpsimd.tensor_tensor(srnv, sigv, rnv, op=A.mult)
                st["srn%d" % a] = srn

            def bg_k(h, last):
                i, a = h // 2, h % 2
                st = kst[i]
                sig, srn = st["sig"], st["srn%d" % a]
                for g in range(8):
                    q = a * 8 + g
                    nc.tensor.matmul(
                        psG[:],
                        srn[:, g * 64 : (g + 1) * 64],
                        sig[:, q * 64 : (q + 1) * 64],
                        start=first_g[0],
                        stop=(last and g == 7),
                    )
                    first_g[0] = False

            for h in range(NH):
                i, a = h // 2, h % 2
                if a == 0:
                    front_k(i)
                bd_k(h)
                if h >= 2:
                    srn_k(h - 2)
                    bg_k(h - 2, last=False)
            srn_k(NH - 2)
            bg_k(NH - 2, last=False)
            srn_k(NH - 1)
            bg_k(NH - 1, last=True)

            # ---- fold G@(-M) into psA, then AllReduce [64,65] ----
            G16 = cpool.tile([64, 64], f16)
            nc.vector.tensor_copy(G16[:], psG[:])
            nc.tensor.matmul(
                psA[:, 0:64], G16[:], negM16[:], start=False, stop=True
            )
            accsb = cpool.tile([64, 65], f32)
            nc.vector.tensor_copy(accsb[:], psA[:])
            if n_cores > 1:
                arin = dram.tile([64, 65], f32)
                arout = dram.tile([64, 65], f32)
                nc.gpsimd.dma_start(arin[:], accsb[:])
                nc.gpsimd.collective_compute(
                    "AllReduce",
                    mybir.AluOpType.add,
                    replica_groups=[list(range(n_cores))],
                    ins=[arin.opt()],
                    outs=[arout.opt()],
                )
                arsb = cpool.tile([64, 65], f32)
            else:
                arsb = accsb

            def update_math():
                # delta_m = clip((sv - G@M)/(B*S), +-1); M' = clip(M+dm,+-100)
                nc.vector.tensor_scalar(
                    mzn[:, 0:64], arsb[:, 0:64], 1.0 / (B * S), MAX_DELTA,
                    op0=A.mult, op1=A.min,
                )
                nc.vector.scalar_tensor_tensor(
                    mzn[:, 0:64], mzn[:, 0:64], -MAX_DELTA, mz[:, 0:64],
                    op0=A.max, op1=A.add,
                )
                nc.vector.tensor_scalar(
                    mzn[:, 0:64], mzn[:, 0:64], MAX_MEMORY, -MAX_MEMORY,
                    op0=A.min, op1=A.max,
                )
                # delta_z = acc_z/B; z' = clip(z+dz, eps, 1e6)
                nc.vector.scalar_tensor_tensor(
                    mzn[:, 64:65], arsb[:, 64:65], 1.0 / B, mz[:, 64:65],
                    op0=A.mult, op1=A.add,
                )
                nc.vector.tensor_scalar(
                    mzn[:, 64:65], mzn[:, 64:65], EPS, MAX_NORM,
                    op0=A.max, op1=A.min,
                )

            def update_cast():
                nc.gpsimd.tensor_copy(Mn2x16[0:64, 0:64], mzn[:, 0:64])
                nc.gpsimd.tensor_copy(Mn2x16[64:128, 64:128], mzn[:, 0:64])
                nc.gpsimd.tensor_copy(zn2x16[0:64, 0:1], mzn[:, 64:65])
                nc.gpsimd.tensor_copy(zn2x16[64:128, 1:2], mzn[:, 64:65])

            # ---------------- queries phase ----------------
            qst = {}
            sigTq = {}

            def front_q(h, allow_gpsimd):
                i, a = h // 2, h % 2
                if a == 0:
                    qt = io.tile([128, MFD], f16, tag="kt")
                    nc.sync.dma_start(qt[:], qr[i])
                    e16 = work.tile([128, MFD], f16, tag="e")
                    nc.scalar.activation(e16[:], qt[:], F.Exp)
                    r16 = work.tile([128, MFD], f16, tag="r")
                    nc.scalar.activation(r16[:], qt[:], F.Relu)
                    sig = work.tile([128, MFD], f16, tag="sig")
                    nc.vector.scalar_tensor_tensor(
                        sig[:], e16[:], 1.0, r16[:], op0=A.min, op1=A.add
                    )
                    qst[i] = {"sig": sig}
                sig = qst[i]["sig"]
                psT = psTp.tile([128, 512], f16, tag="psT")
                for c in range(4):
                    nc.tensor.matmul(
                        psT[:, c * 128 : (c + 1) * 128],
                        sig[:, a * 512 + c * 128 : a * 512 + (c + 1) * 128],
                        ident[:],
                        is_transpose=True,
                        start=(c == 0),
                        stop=(c == 3),
                    )
                sigT = sigq.tile([128, 512], f16, tag="sigTq")
                if ncopy[0] % 2:
                    nc.scalar.copy(sigT[:], psT[:])
                else:
                    nc.vector.tensor_copy(sigT[:], psT[:])
                ncopy[0] += 1
                sigTq[h] = sigT

            def mid_q(h):
                i, a = h // 2, h % 2
                st = qst[i]
                sigT = sigTq[h]
                psR = psRp.tile([128, 512], f32, tag="psR", name="psR")
                st["psR%d" % a] = psR
                if a == 0:
                    st["psN"] = psNp.tile(
                        [128, 16], f32, tag="psN", name="psNq"
                    )
                psN = st["psN"]
                for c in range(4):
                    blk = sigT[:, c * 128 : (c + 1) * 128]
                    nc.tensor.matmul(
                        psR[:, c * 128 : (c + 1) * 128],
                        blk,
                        Mn2x16[:],
                        start=(c == 0),
                        stop=(c == 3),
                    )
                    # n=2 norm matmul reuses the stationary weights just
                    # loaded by the retrieve matmul above.
                    nc.tensor.matmul(
                        psN[:, a * 8 + c * 2 : a * 8 + c * 2 + 2],
                        blk,
                        zn2x16[:],
                        start=(a == 0 and c == 0),
                        stop=(a == 1 and c == 3),
                    )

            def om_q(h):
                i, a = h // 2, h % 2
                st = qst[i]
                if a == 0:
                    rn = small.tile([128, 16], f32, tag="rn")
                    nc.vector.reciprocal(rn[:], st["psN"][:])
                    st["rn"] = rn
                    st["ot"] = io.tile([128, MFD], f16, tag="ot", name="ot")
                rn, ot = st["rn"], st["ot"]
                rnv = (
                    rn[:, a * 8 : (a + 1) * 8]
                    .unsqueeze(2)
                    .broadcast_to((128, 8, 64))
                )
                psRv = st["psR%d" % a][:].rearrange("p (g c) -> p g c", g=8)
                otv = ot[:, a * 512 : (a + 1) * 512].rearrange(
                    "p (g c) -> p g c", g=8
                )
                nc.vector.tensor_mul(otv, psRv, rnv)
                if a == 1:
                    nc.sync.dma_start(orr[i], ot[:])

            def macro_mid(j):
                mid_q(2 * j)
                mid_q(2 * j + 1)
                om_q(2 * j)
                om_q(2 * j + 1)

            SKEW = min(14, NH)
            for h in range(SKEW):
                front_q(h, allow_gpsimd=False)
            if n_cores > 1:
                nc.gpsimd.dma_start(arsb[:], arout[:])
            update_math()
            update_cast()
            mids = 0
            for h in range(SKEW, NH):
                front_q(h, allow_gpsimd=True)
                hh = h - SKEW
                if hh % 2 == 1:
                    macro_mid(hh // 2)
                    mids += 1
            for j in range(mids, NT):
                macro_mid(j)

    nc.compile()
    return nc


_CACHE = {}


def _get_kernel(n_cores, tokens_per_core):
    key = (n_cores, tokens_per_core)
    if key not in _CACHE:
        _CACHE[key] = _build(n_cores, tokens_per_core)
    return _CACHE[key]


def make_in_maps(queries, keys, values, M, z, n_cores=N_CORES):
    b, s, d = keys.shape
    tot = b * s
    tpc = tot // n_cores
    kf = np.ascontiguousarray(keys.reshape(tot, d), dtype=np.float16)
    vf = np.ascontiguousarray(values.reshape(tot, d), dtype=np.float16)
    qf = np.ascontiguousarray(queries.reshape(tot, d), dtype=np.float16)
    m32 = np.ascontiguousarray(M, dtype=np.float32)
    z32 = np.ascontiguousarray(z, dtype=np.float32).reshape(d, 1)
    in_maps = []
    for c in range(n_cores):
        sl = slice(c * tpc, (c + 1) * tpc)
        in_maps.append(
            {
                "keys": np.ascontiguousarray(kf[sl]),
                "values": np.ascontiguousarray(vf[sl]),
                "queries": np.ascontiguousarray(qf[sl]),
                "m": m32,
                "z": z32,
            }
        )
    return in_maps, tpc


def _np_reference(queries, keys, values, M, z):
    """Fallback (is_empty edge case) — straight numpy port of the reference."""

    def elu1(x):
        return np.where(x > 0, x + 1.0, np.exp(np.minimum(x, 0.0)))

    def retrieve(sig, M, z):
        return (sig @ M) / ((sig @ z)[..., None] + EPS)

    sk = elu1(keys)
    existing = retrieve(sk, M, z)
    uv = values if z.sum() == 0 else values - existing
    dm = np.clip(
        np.einsum("bsd,bse->de", sk, uv) / (B * S), -MAX_DELTA, MAX_DELTA
    )
    dz = sk.sum(axis=(0, 1)) / B
    Mn = np.clip(M + dm, -MAX_MEMORY, MAX_MEMORY)
    zn = np.clip(z + dz, EPS, MAX_NORM)
    return retrieve(elu1(queries), Mn, zn).astype(np.float32)


def kernel(queries, keys, values, M, z, _want_results_obj=False, **_ignored):
    from concourse import bass_utils

    queries = np.asarray(queries)
    keys = np.asarray(keys)
    values = np.asarray(values)
    M = np.ascontiguousarray(M, dtype=np.float32)
    z = np.ascontiguousarray(z, dtype=np.float32)

    if float(z.sum()) == 0.0:
        # is_empty branch of the reference: update_values = values. Rare
        # (z all-zero); handled on host rather than in the kernel.
        return _np_reference(
            np.asarray(queries, dtype=np.float32),
            np.asarray(keys, dtype=np.float32),
            np.asarray(values, dtype=np.float32),
            M,
            z,
        )

    b, s, d = keys.shape
    in_maps, tpc = make_in_maps(queries, keys, values, M, z)
    nc = _get_kernel(N_CORES, tpc)
    res = bass_utils.run_bass_kernel_spmd(
        nc, in_maps, core_ids=list(range(N_CORES))
    )
    out = np.concatenate(
        [res.results[c]["out"] for c in range(N_CORES)], axis=0
    ).astype(np.float32).reshape(b, s, d)
    if _want_results_obj:
        return out, res
    return out
